# revision 1
# baseline (speedup 1.0000x reference)
"""AdaptiveDiffusionLayer on 8 TRN2 NeuronCores.

out = (1 - t) * support + t * (adj @ support),  support = x @ weight

Strategy (1D column-parallel SpMM + chunked ReduceScatter):
  - Column-shard adj across 8 cores: core c holds adj[:, c*1250:(c+1)*1250],
    pre-transposed + bf16-cast on the host so the contraction index k lands
    on the SBUF partition axis with unit-stride DMA. Shard x by the SAME k
    rows, so each core's needed support block (support_c = x_c @ W) is
    entirely LOCAL — no all-gather on the critical path.
  - Each core computes partial[i, :] = adj[i, own k] @ support_c for ALL
    10000 output rows, in 10 batches of 8 PSUM accumulators (125-row
    i-tiles, 10 k-tiles of 125 each). Partials are cast to bf16 and
    reduce-scattered in 5 pipelined chunks (the adjT columns are permuted
    host-side so each chunk's rank-r slice is exactly rank r's own output
    rows). Compute of chunk g+1 overlaps the ReduceScatter of chunk g, and
    multi-core launch skew is absorbed by compute instead of idling the PE.
  - Fused epilogue per chunk: out = t * rs_sum + (1-t) * support_c.
"""

import sys

for _p in ("/opt/trn_rl_repo",):
    if _p not in sys.path:
        sys.path.append(_p)

import numpy as np
import ml_dtypes

from concourse import bass, bacc, mybir, tile
from concourse.bass_utils import run_bass_kernel_spmd

N = 10000
IN_F = 512
OUT_F = 512
C = 8               # cores
R = N // C          # 1250 k-rows (adj columns / x rows) per core
SUB = 125           # i-tile rows / k-tile rows
NQ = R // SUB       # 10 local k-tiles
NB = 8              # PSUM accumulators (i-tiles) per batch
BATCH = NB * SUB    # 1000 output rows per batch
NBATCH = N // BATCH  # 10 batches
CHUNK_BATCHES = [2, 2, 2, 2, 1, 1]   # batches per ReduceScatter chunk
G = len(CHUNK_BATCHES)
CHUNK_START = [sum(CHUNK_BATCHES[:g]) for g in range(G)]  # first batch of chunk
BATCH_CHUNK = []                       # batch -> (chunk, index within chunk)
for _g, _n in enumerate(CHUNK_BATCHES):
    for _h in range(_n):
        BATCH_CHUNK.append((_g, _h))

BF16 = mybir.dt.bfloat16
F32 = mybir.dt.float32

_cached = {}


def _build():
    nc = bacc.Bacc("TRN2", target_bir_lowering=False, debug=False, num_devices=C)

    adjt = nc.dram_tensor("adjt", [R, N], BF16, kind="ExternalInput")
    xt = nc.dram_tensor("xt", [IN_F, R], BF16, kind="ExternalInput")
    w = nc.dram_tensor("w", [IN_F, OUT_F], BF16, kind="ExternalInput")
    tsc = nc.dram_tensor("tsc", [128, 2], F32, kind="ExternalInput")
    out = nc.dram_tensor("out", [R, OUT_F], BF16, kind="ExternalOutput")
    ind = nc.dram_tensor("ind", [128, NBATCH * NB], F32, kind="ExternalInput")

    rs_in = [
        nc.dram_tensor(f"rs_in{g}", [CHUNK_BATCHES[g] * BATCH, OUT_F], BF16)
        for g in range(G)
    ]
    rs_out = [
        nc.dram_tensor(f"rs_out{g}", [CHUNK_BATCHES[g] * BATCH // C, OUT_F], BF16)
        for g in range(G)
    ]

    NJ = IN_F // 128  # 4 contraction tiles for x @ W

    with tile.TileContext(nc) as tc:
        with (
            tc.tile_pool(name="persist", bufs=1) as p_pers,
            tc.tile_pool(name="supbf_pool", bufs=1) as p_supbf,
            tc.tile_pool(name="slab_pool", bufs=12) as p_slab,
            tc.tile_pool(name="part_pool", bufs=16) as p_part,
            tc.tile_pool(name="ep_pool", bufs=4) as p_ep,
        ):
            xt_sb = p_pers.tile([128, NJ * R], BF16, tag="xt_sb", name="xt_sb")
            w_sb = p_pers.tile([128, NJ * OUT_F], BF16, tag="w_sb", name="w_sb")
            tsc_sb = p_pers.tile([128, 2], F32, tag="tsc_sb", name="tsc_sb")

            for j in range(NJ):
                nc.scalar.dma_start(
                    out=xt_sb[:, j * R:(j + 1) * R],
                    in_=xt[j * 128:(j + 1) * 128, :],
                )
                nc.scalar.dma_start(
                    out=w_sb[:, j * OUT_F:(j + 1) * OUT_F],
                    in_=w[j * 128:(j + 1) * 128, :],
                )
            nc.scalar.dma_start(out=tsc_sb[:, :], in_=tsc[:, :])
            ind_sb = p_pers.tile(
                [128, NBATCH * NB], F32, tag="ind_sb", name="ind_sb"
            )
            nc.scalar.dma_start(out=ind_sb[:, :], in_=ind[:, :])

            # ---- support_c = x_c @ W, 10 k-subtiles of 125 rows (all local) ----
            supbf = []
            supsc = []
            with tc.tile_pool(name="psum_sup", bufs=3, space="PSUM") as pp_sup:
                for s in range(NQ):
                    ps = pp_sup.tile([SUB, OUT_F], F32, tag="ps", name=f"ps{s}")
                    for j in range(NJ):
                        nc.tensor.matmul(
                            ps[:, :],
                            lhsT=xt_sb[:, j * R + s * SUB: j * R + (s + 1) * SUB],
                            rhs=w_sb[:, j * OUT_F:(j + 1) * OUT_F],
                            start=(j == 0),
                            stop=(j == NJ - 1),
                        )
                    sb = p_supbf.tile(
                        [SUB, OUT_F], BF16, tag=f"supbf{s}", name=f"supbf{s}"
                    )
                    nc.vector.tensor_scalar_mul(
                        sb[:, :], ps[:, :], tsc_sb[0:SUB, 0:1]
                    )
                    supbf.append(sb)
                    sc = p_supbf.tile(
                        [SUB, OUT_F], F32, tag=f"supsc{s}", name=f"supsc{s}"
                    )
                    nc.vector.tensor_scalar_mul(
                        sc[:, :], ps[:, :], tsc_sb[0:SUB, 1:2]
                    )
                    supsc.append(sc)

            # ---- main SpMM: 10 batches x 8 PSUM accumulators over 10
            # k-tiles; every chunk of batches feeds one pipelined
            # ReduceScatter. The matmul operand is pre-scaled by t and each
            # core folds its (1-t)*support term into its OWN rows' partials
            # via a host-supplied 0/1 indicator, so the ReduceScatter output
            # IS the final bf16 result: the epilogue is a pure DRAM->DRAM
            # copy and no compute engine is ever gated on a collective. ----
            with tc.tile_pool(name="psum_main", bufs=1, space="PSUM") as pp_main:
                for b in range(NBATCH):
                    g, h = BATCH_CHUNK[b]
                    acc = [
                        pp_main.tile(
                            [SUB, OUT_F], F32, tag=f"acc{it}", name=f"acc{b}_{it}"
                        )
                        for it in range(NB)
                    ]
                    for qp in range(NQ // 2):
                        slab = p_slab.tile(
                            [SUB, 2 * BATCH], BF16, tag="slab",
                            name=f"slab{b}_{qp}",
                        )
                        nc.sync.dma_start(
                            out=slab[:, :].rearrange("b (a c) -> b a c", a=2),
                            in_=adjt[2 * qp * SUB:(2 * qp + 2) * SUB,
                                     b * BATCH:(b + 1) * BATCH]
                            .rearrange("(a b) c -> b a c", a=2),
                        )
                        for qh in range(2):
                            q = 2 * qp + qh
                            for it in range(NB):
                                nc.tensor.matmul(
                                    acc[it][:, :],
                                    lhsT=slab[:, qh * BATCH + it * SUB:
                                              qh * BATCH + (it + 1) * SUB],
                                    rhs=supbf[q][:, :],
                                    start=(q == 0),
                                    stop=(q == NQ - 1),
                                )
                    for it in range(NB):
                        st = (h * NB + it) % CHUNK_BATCHES[g]
                        s = CHUNK_START[g] + st
                        pt = p_part.tile(
                            [SUB, OUT_F], BF16, tag="pt", name=f"pt{b}_{it}"
                        )
                        col = b * NB + it
                        nc.vector.scalar_tensor_tensor(
                            pt[:, :],
                            supsc[s][:, :],
                            ind_sb[0:SUB, col:col + 1],
                            acc[it][:, :],
                            mybir.AluOpType.mult,
                            mybir.AluOpType.add,
                        )
                        nc.gpsimd.dma_start(
                            out=rs_in[g][h * BATCH + it * SUB:
                                         h * BATCH + (it + 1) * SUB, :],
                            in_=pt[:, :],
                        )
                    if h == CHUNK_BATCHES[g] - 1:
                        nc.gpsimd.collective_compute(
                            "ReduceScatter",
                            mybir.AluOpType.add,
                            replica_groups=[list(range(C))],
                            ins=[rs_in[g].ap().opt()],
                            outs=[rs_out[g].ap().opt()],
                        )
                        nc.scalar.dma_start(
                            out=out[CHUNK_START[g] * SUB:
                                    (CHUNK_START[g] + CHUNK_BATCHES[g]) * SUB,
                                    :],
                            in_=rs_out[g][:, :],
                        )

    nc.compile()
    return nc


def _i_perm():
    """Output-row permutation matching the chunked ReduceScatter layout:
    chunk g covers rows [r*R + off_g, r*R + off_g + len_g) for each rank r,
    in rank order, so each chunk's rank-r slice is rank r's own rows."""
    perm = np.empty(N, np.int64)
    pos = 0
    for g in range(G):
        off = CHUNK_START[g] * SUB
        ln = CHUNK_BATCHES[g] * SUB
        for r in range(C):
            base = r * R + off
            perm[pos:pos + ln] = np.arange(base, base + ln)
            pos += ln
    return perm


def _shard_inputs(x, adj, t, weight):
    bf16 = ml_dtypes.bfloat16
    w_bf = np.asarray(weight, np.float32).astype(bf16)
    t0 = float(np.asarray(t, np.float32).reshape(-1)[0])
    tsc = np.empty((128, 2), np.float32)
    tsc[:, 0] = t0
    tsc[:, 1] = 1.0 - t0

    x = np.asarray(x, np.float32)
    adj = np.asarray(adj, np.float32)
    perm = _i_perm()
    adj_p = adj[perm]  # permuted output rows

    in_maps = []
    for c in range(C):
        cols = slice(c * R, (c + 1) * R)
        adjt = np.ascontiguousarray(adj_p[:, cols].T).astype(bf16)  # [R, N]
        xt = np.ascontiguousarray(x[cols].T).astype(bf16)           # [IN_F, R]
        indv = np.zeros((128, NBATCH * NB), np.float32)
        for b in range(NBATCH):
            g, h = BATCH_CHUNK[b]
            for it in range(NB):
                if (h * NB + it) // CHUNK_BATCHES[g] == c:
                    indv[:, b * NB + it] = 1.0
        in_maps.append(
            {"adjt": adjt, "xt": xt, "w": w_bf, "tsc": tsc, "ind": indv}
        )
    return in_maps


def kernel(x, adj, t, weight):
    if "nc" not in _cached:
        _cached["nc"] = _build()
    nc = _cached["nc"]
    in_maps = _shard_inputs(x, adj, t, weight)
    res = run_bass_kernel_spmd(nc, in_maps, list(range(C)))
    return np.concatenate(
        [res.results[c]["out"].astype(np.float32) for c in range(C)], axis=0
    )

